# revision 16
# baseline (speedup 1.0000x reference)
"""Trainium2 Bass kernel: bipartite GNN message passing (BranchingGNN), 8-core SPMD.

Sharding: core k owns constraint rows [k*6250,(k+1)*6250) and variable rows
[k*12500,(k+1)*12500); each core processes all edges targeting its shard, so
messages need no cross-core reduction. Node tables live row-major in DRAM
(bf16 features in the first 128B of a 256B-strided row) and are re-broadcast
each phase by an AllGather of the updated shards.

Per phase (one message direction):
  - dsts grouped into quads (4 dst-blocks = 512 psum columns); edges sorted
    by (quad, src-window, dst) and cut into raw 128-edge tiles (up to 7 per
    gather call). No slot structure; per-core shortfalls are pad slots that
    re-fetch one of the call's own rows (spread, no hot line) with pdst=-1,
    so every descriptor is a real 128B row read (descriptor count ~= edges,
    vs ~1.3x with slot padding, and no hot dummy row serializing HBM).
  - dma_gather (custom emit: 128B rows at 256B stride) fetches source rows
    row-major [128 edges, 64] bf16.
  - per tile: DVE is_equal(iota, pdst) builds a narrow one-hot S
    [128, sw<=128] bf16 over the tile's dst span; one PE matmul (lhsT=g,
    rhs=S) accumulates into the quad's PSUM [64,512] msgT at column d0
    (the quad's psum is zero-initialized by one matmul against a zero S).
  - per quad: relu(h_prevT + W.T @ msgT + b) in transposed layout, PE
    transpose per block back to row-major into the writeback stage; after
    the last quad one DMA + AllGather republishes the updated shard.

The kernel is bound by SWDGE descriptor generation on the gpsimd queue
pairs (~9.6ns/idx ucode, 4 queue pairs in parallel); PE/DVE reduction and
the collectives fit underneath/between.
"""
import sys

sys.path.insert(0, "/opt/trn_rl_repo")

import numpy as np
import ml_dtypes

import concourse.bass as bass
import concourse.bacc as bacc
import concourse.mybir as mybir
import concourse.tile as tile
from concourse.bass_utils import run_bass_kernel_spmd

# ---- problem constants
V, C, E = 100000, 50000, 1250000
VF, CF, H = 32, 32, 64
ROUNDS = 3
CORES = 8
P = 128
QW = 512              # psum columns per quad (4 blocks)
TPC = 7               # tiles per gather call (SWDGE ring cap)
ROWB = 128            # table row width in bf16 elems (64 data + 64 pad = 256B)

V_CORE, C_CORE = 12500, 6250          # real nodes per core
V_S, C_S = 12672, 6400                # shard rows (99 / 50 blocks)
NBU_V, NBU_C = 98, 49                 # updated blocks (last block stays zero)
RV, RC = CORES * V_S, CORES * C_S     # 101376 / 51200 table rows
VWROWS, CWROWS = 2 * V_S, 4 * C_S     # 25344 / 25600 rows per window

BF16 = mybir.dt.bfloat16
F32 = mybir.dt.float32
I16 = mybir.dt.int16
BF = ml_dtypes.bfloat16

# quads: (chunk, nb). V-dst: chunk0 blocks 0..49, chunk1 50..97.
QLIST_V = [(0, 4)] * 12 + [(0, 2)] + [(1, 4)] * 12
# C-dst: chunk0 blocks 0..24, chunk1 25..48.
QLIST_C = [(0, 4)] * 6 + [(0, 1)] + [(1, 4)] * 6


def _win_var(src):
    """var id -> (window, window-local row): windows of 2 shards."""
    k = src // V_CORE
    l = src % V_CORE
    w = k // 2
    widx = (k % 2) * V_S + l
    return w, widx


def _win_con(src):
    """constr id -> (window, window-local row): windows of 4 shards."""
    k = src // C_CORE
    l = src % C_CORE
    w = k // 4
    widx = (k % 4) * C_S + l
    return w, widx


def _prep_direction(dst, src, n_dst_core, qlist, win_of, nwin):
    """Per-direction metadata. Edges sorted by (core, quad, window, dst);
    per (quad, window) group cut into raw 128-edge tiles; per-core pad
    slots re-fetch one of the call's own rows with pdst=-1.

    Returns (idx [CORES,128,T*8] i16, pdst_rel [CORES,128,T] bf16,
    Tqw [nq,nwin], d0 [T], sw [T])."""
    dst = np.asarray(dst, np.int64)
    src = np.asarray(src, np.int64)
    nq = len(qlist)
    qstart_blk = np.cumsum([0] + [nb for _, nb in qlist])
    blk2q = np.zeros(qstart_blk[-1], np.int64)
    for qi in range(nq):
        blk2q[qstart_blk[qi]:qstart_blk[qi + 1]] = qi

    core_of = dst // n_dst_core
    d_loc = dst % n_dst_core
    b_of = d_loc // P
    q_of = blk2q[b_of]
    dq = d_loc - qstart_blk[q_of] * P                # 0..nb*128-1
    w_of, widx = win_of(src)

    key = ((core_of * nq + q_of) * nwin + w_of) * QW + dq
    order = np.argsort(key, kind="stable")
    ks = key[order]
    widx_s = widx[order]
    dq_s = dq[order]

    gk = ks // QW                                    # (core, q, w) group id
    counts = np.bincount(gk, minlength=CORES * nq * nwin) \
        .reshape(CORES, nq, nwin)
    Tqw = -(-counts.max(0) // P)                     # [nq, nwin] max tiles

    grp_base = np.zeros((nq, nwin), np.int64)
    flat = Tqw.reshape(-1)
    grp_base.reshape(-1)[1:] = np.cumsum(flat)[:-1]
    Ttot = int(flat.sum())

    gcounts = np.bincount(gk, minlength=CORES * nq * nwin)
    gstart = np.zeros(gcounts.size + 1, np.int64)
    gstart[1:] = np.cumsum(gcounts)
    rank = np.arange(dst.size, dtype=np.int64) - gstart[gk]

    c_s = gk // (nq * nwin)
    qw = gk % (nq * nwin)
    epos = grp_base.reshape(-1)[qw] * P + rank

    idx16 = np.full((CORES, Ttot * P), -1, np.int16)
    idx16[c_s, epos] = widx_s.astype(np.int16)
    pdst = np.full((CORES, Ttot * P), -1.0, np.float32)
    pdst[c_s, epos] = dq_s.astype(np.float32)

    # fill pad slots: cycle each call's pads over its own valid idxs
    flatT = Tqw.reshape(-1)
    flat_base = grp_base.reshape(-1)
    for qwi in np.nonzero(flatT > 0)[0]:
        base, T = int(flat_base[qwi]), int(flatT[qwi])
        q_, w_ = qwi // nwin, qwi % nwin
        for j0 in range(0, T, TPC):
            clen = min(TPC, T - j0) * P
            cpos = (base + j0) * P
            for k in range(CORES):
                valid = min(max(int(counts[k, q_, w_]) - j0 * P, 0), clen)
                if valid == clen:
                    continue
                if valid > 0:
                    npad = clen - valid
                    reps = -(-npad // valid)
                    idx16[k, cpos + valid:cpos + clen] = \
                        np.tile(idx16[k, cpos:cpos + valid], reps)[:npad]
                else:
                    gbase = int(flat_base[qwi]) * P
                    gval = min(int(counts[k, q_, w_]), clen)
                    if gval == 0:
                        idx16[k, cpos:cpos + clen] = 0
                    else:
                        reps = -(-clen // gval)
                        idx16[k, cpos:cpos + clen] = \
                            np.tile(idx16[k, gbase:gbase + gval], reps)[:clen]

    # per-tile dst span [d0, d0+sw) in quad coordinates (shared across cores)
    pd3 = pdst.reshape(CORES, Ttot, P)
    valid = pd3 >= 0
    dmin = np.where(valid, pd3, 1e9).min(axis=(0, 2))
    dmax = np.where(valid, pd3, -1).max(axis=(0, 2))
    d0 = np.clip(dmin, 0, QW - 1).astype(np.int64)
    sw = np.maximum(dmax - d0 + 1, 1).astype(np.int64)
    assert sw.max() <= P, f"tile dst span {sw.max()} exceeds {P}"
    pdst_rel = pd3 - d0[None, :, None]
    pdst_rel[~valid] = -1
    pdst_rel = pdst_rel.transpose(0, 2, 1).astype(np.float32)  # [CORES,P,T]

    packed = np.zeros((CORES, P, Ttot * 8), np.int16)
    for k in range(CORES):
        a = idx16[k].reshape(-1, 16).T               # [16, Ttot*8]
        packed[k] = np.tile(a, (8, 1))
    return packed, pdst_rel.astype(BF), Tqw.astype(int), d0, sw


def _dma_gather_raw(gp, out_ap, in_ap, idxs_ap, num_idxs, elem_size, elem_step,
                    queue_num=0):
    """dma_gather (non-transpose, HBM source) allowing 128B rows at 256B stride."""
    from concourse import ap_utils
    gp._assert_queue_num(queue_num)
    assert idxs_ap.dtype == mybir.dt.int16
    assert in_ap.dtype == out_ap.dtype
    assert ap_utils.ap_is_contiguous(in_ap.ap[1:])
    assert ap_utils.ap_is_contiguous(out_ap.ap[1:])
    assert ap_utils.ap_is_contiguous(idxs_ap.ap[1:])
    assert in_ap.ap[-1][1] == out_ap.ap[-1][1] == elem_size
    assert out_ap.ap[0][1] * out_ap.ap[1][1] == num_idxs and num_idxs % 128 == 0
    assert in_ap.ap[0][0] == elem_step
    stride_bytes = elem_step * mybir.dt.size(in_ap.dtype)
    stride_bytes_256 = stride_bytes // 256
    assert stride_bytes_256 * 256 == stride_bytes and stride_bytes_256 < 256
    _in_ap = gp.lower_ap_dma(in_ap, for_custom_bir_dma=True)
    _idxs_ap = gp.lower_ap(idxs_ap)
    _out_ap = gp.lower_ap(out_ap)
    return gp.add_instruction(
        mybir.InstDMAGatherAnt(
            name=gp.bass.get_next_instruction_name(),
            ins=[*_in_ap, _idxs_ap, gp.lower_val_access(gp.to_reg(num_idxs))],
            outs=[_out_ap],
            transpose=False, num_idxs=num_idxs, elem_size=elem_size,
            stride_bytes_256=stride_bytes_256, gen_mode=0, single_packet=True,
            queue_num=queue_num, sbuf_tokens_per_rank=0,
            sbuf_free_dim_per_rank=0, sbuf_free_dim_pad_per_rank=0,
            sbuf_byte_offset=0))


def _build(Tqw_c, Tqw_v, d0_c, sw_c, d0_v, sw_v, b_score_val):
    """Build the shared SPMD program."""

    Tt_c, Tt_v = int(Tqw_c.sum()), int(Tqw_v.sum())

    nc = bacc.Bacc("TRN2", target_bir_lowering=False, num_devices=CORES,
                   num_swdge_queues=4)
    AluOp = mybir.AluOpType
    Act = mybir.ActivationFunctionType

    def ein(name, shape, dtype):
        return nc.dram_tensor(name, shape, dtype, kind="ExternalInput")

    vfT = ein("vfT", [VF, V_S], F32)
    cfT = ein("cfT", [CF, C_S], F32)
    wvar = ein("wvar", [VF, H], F32)
    wcon = ein("wcon", [CF, H], F32)
    wv2c = ein("wv2c", [H, H], F32)
    wc2v = ein("wc2v", [H, H], F32)
    wsco = ein("wsco", [H, 1], BF16)
    bvar = ein("bvar", [H, 1], F32)
    bcon = ein("bcon", [H, 1], F32)
    bv2c = ein("bv2c", [H, 1], F32)
    bc2v = ein("bc2v", [H, 1], F32)
    idx_v2c_d = ein("idx_v2c", [P, Tt_c * 8], I16)
    idx_c2v_d = ein("idx_c2v", [P, Tt_v * 8], I16)
    pdst_v2c_d = ein("pdst_v2c", [P, Tt_c], BF16)
    pdst_c2v_d = ein("pdst_c2v", [P, Tt_v], BF16)
    iota_d = ein("iota", [P, P], BF16)
    ident_d = ein("ident", [H, H], BF16)
    scores_out = nc.dram_tensor("scores", [V_S], F32, kind="ExternalOutput")

    with tile.TileContext(nc) as tc:
        with (
            tc.tile_pool(name="const", bufs=1) as cpool,
            tc.tile_pool(name="state", bufs=1) as spool,
            tc.tile_pool(name="dram", bufs=1, space="DRAM") as dpool,
            tc.tile_pool(name="gpool", bufs=24) as gpool,
            tc.tile_pool(name="s2p", bufs=6) as s2_pool,
            tc.tile_pool(name="msgp", bufs=2) as msg_pool,
            tc.tile_pool(name="rowp", bufs=2) as row_pool,
            tc.tile_pool(name="ps_acc", bufs=3, space="PSUM") as ps_acc,
            tc.tile_pool(name="ps_upd", bufs=2, space="PSUM") as ps_upd,
            tc.tile_pool(name="ps_misc", bufs=2, space="PSUM") as ps_misc,
        ):
            def load_const(name, dram, shape, dtype):
                t = cpool.tile(shape, dtype, name=name)
                nc.sync.dma_start(out=t[:], in_=dram[:])
                return t

            iota_sb = load_const("iota_sb", iota_d, [P, P], BF16)
            ident_sb = load_const("ident_sb", ident_d, [H, H], BF16)
            wvar_sb = load_const("wvar_sb", wvar, [VF, H], F32)
            wcon_sb = load_const("wcon_sb", wcon, [CF, H], F32)
            wv2c_sb = load_const("wv2c_sb", wv2c, [H, H], F32)
            wc2v_sb = load_const("wc2v_sb", wc2v, [H, H], F32)
            wsco_sb = load_const("wsco_sb", wsco, [H, 1], BF16)
            bvar_sb = load_const("bvar_sb", bvar, [H, 1], F32)
            bcon_sb = load_const("bcon_sb", bcon, [H, 1], F32)
            bv2c_sb = load_const("bv2c_sb", bv2c, [H, 1], F32)
            bc2v_sb = load_const("bc2v_sb", bc2v, [H, 1], F32)
            idx_v2c_sb = load_const("idx_v2c_sb", idx_v2c_d, [P, Tt_c * 8], I16)
            idx_c2v_sb = load_const("idx_c2v_sb", idx_c2v_d, [P, Tt_v * 8], I16)
            pdst_v2c_sb = load_const("pdst_v2c_sb", pdst_v2c_d, [P, Tt_c], BF16)
            pdst_c2v_sb = load_const("pdst_c2v_sb", pdst_c2v_d, [P, Tt_v], BF16)

            zrow_sb = cpool.tile([P, ROWB], BF16, name="zrow_sb")
            nc.vector.memset(zrow_sb[:], 0.0)
            szero_sb = cpool.tile([P, QW], BF16, name="szero_sb")
            nc.vector.memset(szero_sb[:], 0.0)

            hvT = spool.tile([H, V_S], BF16, name="hvT")
            hcT = spool.tile([H, C_S], BF16, name="hcT")
            tabs_v = [dpool.tile([RV, ROWB], BF16, name=f"tab_v{i}",
                                 addr_space="Shared", tag=f"tab_v{i}")
                      for i in range(ROUNDS)]
            tabs_c = [dpool.tile([RC, ROWB], BF16, name=f"tab_c{i}",
                                 addr_space="Shared", tag=f"tab_c{i}")
                      for i in range(ROUNDS)]
            agin_v = dpool.tile([V_S, ROWB], BF16, name="agin_v")
            agin_c = dpool.tile([C_S, ROWB], BF16, name="agin_c")

            # zero the shard tail (pad rows shipped by every AllGather)
            nc.sync.dma_start(out=agin_v[NBU_V * P:V_S, :], in_=zrow_sb[:])
            nc.sync.dma_start(out=agin_c[NBU_C * P:C_S, :], in_=zrow_sb[:])

            # gather windows: (row start, rows) into the direction's table
            WIN_V = [(w * VWROWS, VWROWS) for w in range(4)]
            WIN_C = [(w * CWROWS, CWROWS) for w in range(2)]

            # ---- initial embeddings hT = relu(W.T @ featT + b)
            def emit_init(featT_dram, fdim, n_s, w_sb, b_sb, hT):
                with tc.tile_pool(name="initp", bufs=2) as ipool:
                    c0 = 0
                    while c0 < n_s:
                        w = min(512, n_s - c0)
                        fch = ipool.tile([fdim, 512], F32, name="fch", tag="fch")
                        nc.sync.dma_start(out=fch[:, :w],
                                          in_=featT_dram[:, c0:c0 + w])
                        psi = ps_misc.tile([H, 512], F32, name="psi", tag="misc")
                        nc.tensor.matmul(out=psi[:, :w], lhsT=w_sb[:],
                                         rhs=fch[:, :w], start=True, stop=True)
                        nc.scalar.activation(out=hT[:, c0:c0 + w], in_=psi[:, :w],
                                             func=Act.Relu, bias=b_sb[:])
                        c0 += w

            emit_init(vfT, VF, V_S, wvar_sb, bvar_sb, hvT)
            emit_init(cfT, CF, C_S, wcon_sb, bcon_sb, hcT)

            qctr = [0]

            def emit_writeback(nblk, agin, tab, rstage):
                nc.sync.dma_start(
                    out=agin[0:nblk * P, 0:H]
                    .rearrange("(b p) f -> p b f", p=P),
                    in_=rstage[:, :nblk * H]
                    .rearrange("p (b f) -> p b f", f=H))
                nc.gpsimd.collective_compute(
                    "AllGather", mybir.AluOpType.bypass,
                    replica_groups=[list(range(CORES))],
                    ins=[agin[:]], outs=[tab[:]])

            def emit_shard_publish(hT, nblk, agin, tab):
                rstage = row_pool.tile([P, NBU_V * H], BF16, name="rstage",
                                       tag="rstage")
                for b in range(nblk):
                    psr = ps_misc.tile([P, H], BF16, name="psr", tag="misc")
                    nc.tensor.transpose(out=psr[:], in_=hT[:, b * P:(b + 1) * P],
                                        identity=ident_sb[:])
                    nc.vector.tensor_copy(out=rstage[:, b * H:(b + 1) * H],
                                          in_=psr[:])
                emit_writeback(nblk, agin, tab, rstage)

            emit_shard_publish(hvT, NBU_V, agin_v, tabs_v[0])

            # ---- one message-passing phase
            def emit_phase(tab_src, wins, idx_sb, pdst_sb, Tqw, d0a, swa,
                           qlist, hT, W_sb, b_sb, wb):
                nwin = Tqw.shape[1]
                grp_base = np.zeros((len(qlist), nwin), np.int64)
                grp_base.reshape(-1)[1:] = np.cumsum(Tqw.reshape(-1))[:-1]
                rstage = row_pool.tile([P, NBU_V * H], BF16, name="rstage",
                                       tag="rstage")
                qb = 0
                for qi, (chk, nb) in enumerate(qlist):
                    nmm = int(Tqw[qi].sum())
                    ps = ps_acc.tile([H, QW], F32, name="ps", tag="ps")
                    nc.tensor.matmul(out=ps[:, :nb * P], lhsT=zrow_sb[:, :H],
                                     rhs=szero_sb[:, :nb * P], start=True,
                                     stop=(nmm == 0))
                    mm = 0
                    for w in range(nwin):
                        Tg = int(Tqw[qi, w])
                        base = int(grp_base[qi, w])
                        r0, rws = wins[w]
                        done = 0
                        while done < Tg:
                            tc_ = min(TPC, Tg - done)
                            g = gpool.tile([P, TPC, H], BF16, name="g", tag="g")
                            _dma_gather_raw(
                                nc.gpsimd, g[:, :tc_, :],
                                tab_src[r0:r0 + rws, 0:H],
                                idx_sb[:, (base + done) * 8:
                                       (base + done + tc_) * 8],
                                num_idxs=tc_ * P, elem_size=H, elem_step=ROWB,
                                queue_num=qctr[0] % 4)
                            qctr[0] += 1
                            for t in range(tc_):
                                gt = base + done + t
                                d0_, sw_ = int(d0a[gt]), int(swa[gt])
                                s2 = s2_pool.tile([P, P], BF16, name="s2",
                                                  tag="s2")
                                nc.vector.tensor_tensor(
                                    out=s2[:, :sw_], in0=iota_sb[:, :sw_],
                                    in1=pdst_sb[:, gt:gt + 1]
                                    .to_broadcast([P, sw_]),
                                    op=AluOp.is_equal)
                                nc.tensor.matmul(
                                    out=ps[:, d0_:d0_ + sw_],
                                    lhsT=g[:, t, :], rhs=s2[:, :sw_],
                                    start=False, stop=(mm == nmm - 1))
                                mm += 1
                            done += tc_
                    # quad update: hT = relu(hT + W.T @ msgT + b)
                    cw = nb * P
                    msg = msg_pool.tile([H, QW], F32, name="msg", tag="msg")
                    nc.vector.tensor_copy(out=msg[:, :cw], in_=ps[:, :cw])
                    psu = ps_upd.tile([H, QW], F32, name="psu", tag="psu")
                    nc.tensor.matmul(out=psu[:, :cw], lhsT=W_sb[:],
                                     rhs=msg[:, :cw], start=True, stop=True)
                    tmp = msg_pool.tile([H, QW], F32, name="tmp", tag="tmp")
                    nc.vector.tensor_tensor(out=tmp[:, :cw], in0=psu[:, :cw],
                                            in1=hT[:, qb * P:qb * P + cw],
                                            op=AluOp.add)
                    nc.scalar.activation(out=hT[:, qb * P:qb * P + cw],
                                         in_=tmp[:, :cw],
                                         func=Act.Relu, bias=b_sb[:])
                    if wb is not None:
                        for j in range(nb):
                            b = qb + j
                            psr = ps_misc.tile([P, H], BF16, name="psr",
                                               tag="misc")
                            nc.tensor.transpose(
                                out=psr[:], in_=hT[:, b * P:(b + 1) * P],
                                identity=ident_sb[:])
                            nc.vector.tensor_copy(
                                out=rstage[:, b * H:(b + 1) * H], in_=psr[:])
                    qb += nb
                if wb is not None:
                    emit_writeback(qb, wb[0], wb[1], rstage)

            for r in range(ROUNDS):
                emit_phase(tabs_v[r], WIN_V, idx_v2c_sb, pdst_v2c_sb, Tqw_c,
                           d0_c, sw_c, QLIST_C, hcT, wv2c_sb, bv2c_sb,
                           (agin_c, tabs_c[r]))
                last = r == ROUNDS - 1
                emit_phase(tabs_c[r], WIN_C, idx_c2v_sb, pdst_c2v_sb, Tqw_v,
                           d0_v, sw_v, QLIST_V, hvT, wc2v_sb, bc2v_sb,
                           None if last else (agin_v, tabs_v[r + 1]))

            # ---- scores = h_var @ w_score + b_score (shard)
            c0 = 0
            while c0 < V_S:
                w = min(512, V_S - c0)
                pss = ps_misc.tile([1, 512], F32, name="pss", tag="misc")
                nc.tensor.matmul(out=pss[:, :w], lhsT=wsco_sb[:],
                                 rhs=hvT[:, c0:c0 + w], start=True, stop=True)
                sch = s2_pool.tile([1, 512], F32, name="sch", tag="sch")
                nc.vector.tensor_scalar(
                    out=sch[:, :w], in0=pss[:, :w],
                    scalar1=float(b_score_val), scalar2=None, op0=AluOp.add)
                nc.sync.dma_start(out=scores_out[None, c0:c0 + w],
                                  in_=sch[0:1, :w])
                c0 += 512

    nc.compile()
    return nc


_CACHE = {}


def kernel(**inputs):
    var_feat = np.asarray(inputs["var_feat"], np.float32)
    constr_feat = np.asarray(inputs["constr_feat"], np.float32)
    var_idx = np.asarray(inputs["var_idx"]).astype(np.int64)
    constr_idx = np.asarray(inputs["constr_idx"]).astype(np.int64)
    b_score_val = float(np.asarray(inputs["b_score"]).reshape(-1)[0])

    key = (var_idx.tobytes(), constr_idx.tobytes())
    if key in _CACHE:
        nc, idx_v, pdst_v, idx_c, pdst_c = _CACHE[key]
    else:
        # v2c: dst=constr, src=var (gathers V table)
        idx_v, pdst_v, Tqw_c, d0_c, sw_c = _prep_direction(
            constr_idx, var_idx, C_CORE, QLIST_C, _win_var, 4)
        # c2v: dst=var, src=constr (gathers C table)
        idx_c, pdst_c, Tqw_v, d0_v, sw_v = _prep_direction(
            var_idx, constr_idx, V_CORE, QLIST_V, _win_con, 2)
        nc = _build(Tqw_c, Tqw_v, d0_c, sw_c, d0_v, sw_v, b_score_val)
        _CACHE[key] = (nc, idx_v, pdst_v, idx_c, pdst_c)

    iota = np.broadcast_to(np.arange(P, dtype=np.float32),
                           (P, P)).astype(BF).copy()
    ident = np.eye(H, dtype=np.float32).astype(BF)

    vf_pad = np.zeros((CORES, V_S, VF), np.float32)
    vf_pad[:, :V_CORE] = var_feat.reshape(CORES, V_CORE, VF)
    cf_pad = np.zeros((CORES, C_S, CF), np.float32)
    cf_pad[:, :C_CORE] = constr_feat.reshape(CORES, C_CORE, CF)

    common = dict(
        wvar=np.ascontiguousarray(inputs["W_var"], dtype=np.float32),
        wcon=np.ascontiguousarray(inputs["W_con"], dtype=np.float32),
        wv2c=np.ascontiguousarray(inputs["W_v2c"], dtype=np.float32),
        wc2v=np.ascontiguousarray(inputs["W_c2v"], dtype=np.float32),
        wsco=np.ascontiguousarray(inputs["W_score"], dtype=np.float32).astype(BF),
        bvar=np.ascontiguousarray(inputs["b_var"], dtype=np.float32).reshape(H, 1),
        bcon=np.ascontiguousarray(inputs["b_con"], dtype=np.float32).reshape(H, 1),
        bv2c=np.ascontiguousarray(inputs["b_v2c"], dtype=np.float32).reshape(H, 1),
        bc2v=np.ascontiguousarray(inputs["b_c2v"], dtype=np.float32).reshape(H, 1),
        iota=iota, ident=ident,
    )
    in_maps = []
    for k in range(CORES):
        m = dict(common)
        m["vfT"] = np.ascontiguousarray(vf_pad[k].T)
        m["cfT"] = np.ascontiguousarray(cf_pad[k].T)
        m["idx_v2c"] = idx_v[k]
        m["pdst_v2c"] = pdst_v[k]
        m["idx_c2v"] = idx_c[k]
        m["pdst_c2v"] = pdst_c[k]
        in_maps.append(m)

    res = run_bass_kernel_spmd(nc, in_maps, list(range(CORES)))
    scores = np.concatenate([res.results[k]["scores"].reshape(-1)[:V_CORE]
                             for k in range(CORES)])
    return scores.astype(np.float32)


# revision 17
# speedup vs baseline: 1.0120x; 1.0120x over previous
"""Trainium2 Bass kernel: bipartite GNN message passing (BranchingGNN), 8-core SPMD.

Sharding: core k owns constraint rows [k*6250,(k+1)*6250) and variable rows
[k*12500,(k+1)*12500); each core processes all edges targeting its shard, so
messages need no cross-core reduction. Node tables live row-major in DRAM
(bf16 features in the first 128B of a 256B-strided row) and are re-broadcast
each phase by an AllGather of the updated shards.

Per phase (one message direction):
  - dsts grouped into quads (4 dst-blocks = 512 psum columns); edges sorted
    by (quad, src-window, dst) and cut into raw 128-edge tiles (up to 7 per
    gather call). No slot structure; per-core shortfalls are pad slots that
    re-fetch one of the call's own rows (spread, no hot line) with pdst=-1,
    so every descriptor is a real 128B row read (descriptor count ~= edges,
    vs ~1.3x with slot padding, and no hot dummy row serializing HBM).
  - dma_gather (custom emit: 128B rows at 256B stride) fetches source rows
    row-major [128 edges, 64] bf16.
  - per tile: DVE is_equal(iota, pdst) builds a narrow one-hot S
    [128, sw<=128] bf16 over the tile's dst span; one PE matmul (lhsT=g,
    rhs=S) accumulates into the quad's PSUM [64,512] msgT at column d0
    (the quad's psum is zero-initialized by one matmul against a zero S).
  - per quad: relu(h_prevT + W.T @ msgT + b) in transposed layout, PE
    transpose per block back to row-major into the writeback stage; after
    the last quad one DMA + AllGather republishes the updated shard.

The kernel is bound by SWDGE descriptor generation on the gpsimd queue
pairs (~9.6ns/idx ucode, 4 queue pairs in parallel); PE/DVE reduction and
the collectives fit underneath/between.
"""
import sys

sys.path.insert(0, "/opt/trn_rl_repo")

import numpy as np
import ml_dtypes

import concourse.bass as bass
import concourse.bacc as bacc
import concourse.mybir as mybir
import concourse.tile as tile
from concourse.bass_utils import run_bass_kernel_spmd

# ---- problem constants
V, C, E = 100000, 50000, 1250000
VF, CF, H = 32, 32, 64
ROUNDS = 3
CORES = 8
P = 128
QW = 512              # psum columns per quad (4 blocks)
TPC = 7               # tiles per gather call (SWDGE ring cap)
ROWB = 128            # table row width in bf16 elems (64 data + 64 pad = 256B)

V_CORE, C_CORE = 12500, 6250          # real nodes per core
V_S, C_S = 12672, 6400                # shard rows (99 / 50 blocks)
NBU_V, NBU_C = 98, 49                 # updated blocks (last block stays zero)
RV, RC = CORES * V_S, CORES * C_S     # 101376 / 51200 table rows
VWROWS, CWROWS = 2 * V_S, 4 * C_S     # 25344 / 25600 rows per window

BF16 = mybir.dt.bfloat16
F32 = mybir.dt.float32
I16 = mybir.dt.int16
BF = ml_dtypes.bfloat16

# quads: (chunk, nb). V-dst: chunk0 blocks 0..49, chunk1 50..97.
QLIST_V = [(0, 4)] * 12 + [(0, 2)] + [(1, 4)] * 12
# C-dst: chunk0 blocks 0..24, chunk1 25..48.
QLIST_C = [(0, 4)] * 6 + [(0, 1)] + [(1, 4)] * 6


def _win_var(src):
    """var id -> (window, window-local row): windows of 2 shards."""
    k = src // V_CORE
    l = src % V_CORE
    w = k // 2
    widx = (k % 2) * V_S + l
    return w, widx


def _win_con(src):
    """constr id -> (window, window-local row): windows of 4 shards."""
    k = src // C_CORE
    l = src % C_CORE
    w = k // 4
    widx = (k % 4) * C_S + l
    return w, widx


def _prep_direction(dst, src, n_dst_core, qlist, win_of, nwin):
    """Per-direction metadata. Edges sorted by (core, quad, window, dst);
    per (quad, window) group cut into raw 128-edge tiles; per-core pad
    slots re-fetch one of the call's own rows with pdst=-1.

    Returns (idx [CORES,128,T*8] i16, pdst_rel [CORES,128,T] bf16,
    Tqw [nq,nwin], d0 [T], sw [T])."""
    dst = np.asarray(dst, np.int64)
    src = np.asarray(src, np.int64)
    nq = len(qlist)
    qstart_blk = np.cumsum([0] + [nb for _, nb in qlist])
    blk2q = np.zeros(qstart_blk[-1], np.int64)
    for qi in range(nq):
        blk2q[qstart_blk[qi]:qstart_blk[qi + 1]] = qi

    core_of = dst // n_dst_core
    d_loc = dst % n_dst_core
    b_of = d_loc // P
    q_of = blk2q[b_of]
    dq = d_loc - qstart_blk[q_of] * P                # 0..nb*128-1
    w_of, widx = win_of(src)

    key = ((core_of * nq + q_of) * nwin + w_of) * QW + dq
    order = np.argsort(key, kind="stable")
    ks = key[order]
    widx_s = widx[order]
    dq_s = dq[order]

    gk = ks // QW                                    # (core, q, w) group id
    counts = np.bincount(gk, minlength=CORES * nq * nwin) \
        .reshape(CORES, nq, nwin)
    Tqw = -(-counts.max(0) // P)                     # [nq, nwin] max tiles

    grp_base = np.zeros((nq, nwin), np.int64)
    flat = Tqw.reshape(-1)
    grp_base.reshape(-1)[1:] = np.cumsum(flat)[:-1]
    Ttot = int(flat.sum())

    gcounts = np.bincount(gk, minlength=CORES * nq * nwin)
    gstart = np.zeros(gcounts.size + 1, np.int64)
    gstart[1:] = np.cumsum(gcounts)
    rank = np.arange(dst.size, dtype=np.int64) - gstart[gk]

    c_s = gk // (nq * nwin)
    qw = gk % (nq * nwin)
    epos = grp_base.reshape(-1)[qw] * P + rank

    idx16 = np.full((CORES, Ttot * P), -1, np.int16)
    idx16[c_s, epos] = widx_s.astype(np.int16)
    pdst = np.full((CORES, Ttot * P), -1.0, np.float32)
    pdst[c_s, epos] = dq_s.astype(np.float32)

    # fill pad slots: cycle each call's pads over its own valid idxs
    flatT = Tqw.reshape(-1)
    flat_base = grp_base.reshape(-1)
    for qwi in np.nonzero(flatT > 0)[0]:
        base, T = int(flat_base[qwi]), int(flatT[qwi])
        q_, w_ = qwi // nwin, qwi % nwin
        for j0 in range(0, T, TPC):
            clen = min(TPC, T - j0) * P
            cpos = (base + j0) * P
            for k in range(CORES):
                valid = min(max(int(counts[k, q_, w_]) - j0 * P, 0), clen)
                if valid == clen:
                    continue
                if valid > 0:
                    npad = clen - valid
                    reps = -(-npad // valid)
                    idx16[k, cpos + valid:cpos + clen] = \
                        np.tile(idx16[k, cpos:cpos + valid], reps)[:npad]
                else:
                    gbase = int(flat_base[qwi]) * P
                    gval = min(int(counts[k, q_, w_]), clen)
                    if gval == 0:
                        idx16[k, cpos:cpos + clen] = 0
                    else:
                        reps = -(-clen // gval)
                        idx16[k, cpos:cpos + clen] = \
                            np.tile(idx16[k, gbase:gbase + gval], reps)[:clen]

    # per-tile dst span [d0, d0+sw) in quad coordinates (shared across cores)
    pd3 = pdst.reshape(CORES, Ttot, P)
    valid = pd3 >= 0
    dmin = np.where(valid, pd3, 1e9).min(axis=(0, 2))
    dmax = np.where(valid, pd3, -1).max(axis=(0, 2))
    d0 = np.clip(dmin, 0, QW - 1).astype(np.int64)
    sw = np.maximum(dmax - d0 + 1, 1).astype(np.int64)
    assert sw.max() <= P, f"tile dst span {sw.max()} exceeds {P}"
    pdst_rel = pd3 - d0[None, :, None]
    pdst_rel[~valid] = -1
    pdst_rel = pdst_rel.transpose(0, 2, 1).astype(np.float32)  # [CORES,P,T]

    packed = np.zeros((CORES, P, Ttot * 8), np.int16)
    for k in range(CORES):
        a = idx16[k].reshape(-1, 16).T               # [16, Ttot*8]
        packed[k] = np.tile(a, (8, 1))
    return packed, pdst_rel.astype(BF), Tqw.astype(int), d0, sw


def _dma_gather_raw(gp, out_ap, in_ap, idxs_ap, num_idxs, elem_size, elem_step,
                    queue_num=0):
    """dma_gather (non-transpose, HBM source) allowing 128B rows at 256B stride."""
    from concourse import ap_utils
    gp._assert_queue_num(queue_num)
    assert idxs_ap.dtype == mybir.dt.int16
    assert in_ap.dtype == out_ap.dtype
    assert ap_utils.ap_is_contiguous(in_ap.ap[1:])
    assert ap_utils.ap_is_contiguous(out_ap.ap[1:])
    assert ap_utils.ap_is_contiguous(idxs_ap.ap[1:])
    assert in_ap.ap[-1][1] == out_ap.ap[-1][1] == elem_size
    assert out_ap.ap[0][1] * out_ap.ap[1][1] == num_idxs and num_idxs % 128 == 0
    assert in_ap.ap[0][0] == elem_step
    stride_bytes = elem_step * mybir.dt.size(in_ap.dtype)
    stride_bytes_256 = stride_bytes // 256
    assert stride_bytes_256 * 256 == stride_bytes and stride_bytes_256 < 256
    _in_ap = gp.lower_ap_dma(in_ap, for_custom_bir_dma=True)
    _idxs_ap = gp.lower_ap(idxs_ap)
    _out_ap = gp.lower_ap(out_ap)
    return gp.add_instruction(
        mybir.InstDMAGatherAnt(
            name=gp.bass.get_next_instruction_name(),
            ins=[*_in_ap, _idxs_ap, gp.lower_val_access(gp.to_reg(num_idxs))],
            outs=[_out_ap],
            transpose=False, num_idxs=num_idxs, elem_size=elem_size,
            stride_bytes_256=stride_bytes_256, gen_mode=0, single_packet=True,
            queue_num=queue_num, sbuf_tokens_per_rank=0,
            sbuf_free_dim_per_rank=0, sbuf_free_dim_pad_per_rank=0,
            sbuf_byte_offset=0))


def _build(Tqw_c, Tqw_v, d0_c, sw_c, d0_v, sw_v, b_score_val):
    """Build the shared SPMD program."""

    Tt_c, Tt_v = int(Tqw_c.sum()), int(Tqw_v.sum())

    nc = bacc.Bacc("TRN2", target_bir_lowering=False, num_devices=CORES,
                   num_swdge_queues=4)
    AluOp = mybir.AluOpType
    Act = mybir.ActivationFunctionType

    def ein(name, shape, dtype):
        return nc.dram_tensor(name, shape, dtype, kind="ExternalInput")

    vfT = ein("vfT", [VF, V_S], F32)
    cfT = ein("cfT", [CF, C_S], F32)
    wvar = ein("wvar", [VF, H], F32)
    wcon = ein("wcon", [CF, H], F32)
    wv2c = ein("wv2c", [H, H], F32)
    wc2v = ein("wc2v", [H, H], F32)
    wsco = ein("wsco", [H, 1], BF16)
    bvar = ein("bvar", [H, 1], F32)
    bcon = ein("bcon", [H, 1], F32)
    bv2c = ein("bv2c", [H, 1], F32)
    bc2v = ein("bc2v", [H, 1], F32)
    idx_v2c_d = ein("idx_v2c", [P, Tt_c * 8], I16)
    idx_c2v_d = ein("idx_c2v", [P, Tt_v * 8], I16)
    pdst_v2c_d = ein("pdst_v2c", [P, Tt_c], BF16)
    pdst_c2v_d = ein("pdst_c2v", [P, Tt_v], BF16)
    iota_d = ein("iota", [P, P], BF16)
    ident_d = ein("ident", [H, H], BF16)
    scores_out = nc.dram_tensor("scores", [V_S], F32, kind="ExternalOutput")

    with tile.TileContext(nc) as tc:
        with (
            tc.tile_pool(name="const", bufs=1) as cpool,
            tc.tile_pool(name="state", bufs=1) as spool,
            tc.tile_pool(name="dram", bufs=1, space="DRAM") as dpool,
            tc.tile_pool(name="gpool", bufs=24) as gpool,
            tc.tile_pool(name="s2p", bufs=6) as s2_pool,
            tc.tile_pool(name="msgp", bufs=2) as msg_pool,
            tc.tile_pool(name="rowp", bufs=2) as row_pool,
            tc.tile_pool(name="ps_acc", bufs=3, space="PSUM") as ps_acc,
            tc.tile_pool(name="ps_upd", bufs=2, space="PSUM") as ps_upd,
            tc.tile_pool(name="ps_misc", bufs=2, space="PSUM") as ps_misc,
        ):
            def load_const(name, dram, shape, dtype):
                t = cpool.tile(shape, dtype, name=name)
                nc.sync.dma_start(out=t[:], in_=dram[:])
                return t

            iota_sb = load_const("iota_sb", iota_d, [P, P], BF16)
            ident_sb = load_const("ident_sb", ident_d, [H, H], BF16)
            wvar_sb = load_const("wvar_sb", wvar, [VF, H], F32)
            wcon_sb = load_const("wcon_sb", wcon, [CF, H], F32)
            wv2c_sb = load_const("wv2c_sb", wv2c, [H, H], F32)
            wc2v_sb = load_const("wc2v_sb", wc2v, [H, H], F32)
            wsco_sb = load_const("wsco_sb", wsco, [H, 1], BF16)
            bvar_sb = load_const("bvar_sb", bvar, [H, 1], F32)
            bcon_sb = load_const("bcon_sb", bcon, [H, 1], F32)
            bv2c_sb = load_const("bv2c_sb", bv2c, [H, 1], F32)
            bc2v_sb = load_const("bc2v_sb", bc2v, [H, 1], F32)
            idx_v2c_sb = load_const("idx_v2c_sb", idx_v2c_d, [P, Tt_c * 8], I16)
            idx_c2v_sb = load_const("idx_c2v_sb", idx_c2v_d, [P, Tt_v * 8], I16)
            pdst_v2c_sb = load_const("pdst_v2c_sb", pdst_v2c_d, [P, Tt_c], BF16)
            pdst_c2v_sb = load_const("pdst_c2v_sb", pdst_c2v_d, [P, Tt_v], BF16)

            zrow_sb = cpool.tile([P, ROWB], BF16, name="zrow_sb")
            nc.vector.memset(zrow_sb[:], 0.0)
            szero_sb = cpool.tile([P, QW], BF16, name="szero_sb")
            nc.vector.memset(szero_sb[:], 0.0)

            hvT = spool.tile([H, V_S], BF16, name="hvT")
            hcT = spool.tile([H, C_S], BF16, name="hcT")
            tabs_v = [dpool.tile([RV, ROWB], BF16, name=f"tab_v{i}",
                                 addr_space="Shared", tag=f"tab_v{i}")
                      for i in range(ROUNDS)]
            tabs_c = [dpool.tile([RC, ROWB], BF16, name=f"tab_c{i}",
                                 addr_space="Shared", tag=f"tab_c{i}")
                      for i in range(ROUNDS)]
            agin_v = dpool.tile([V_S, ROWB], BF16, name="agin_v")
            agin_c = dpool.tile([C_S, ROWB], BF16, name="agin_c")

            # zero the shard tail (pad rows shipped by every AllGather)
            nc.sync.dma_start(out=agin_v[NBU_V * P:V_S, :], in_=zrow_sb[:])
            nc.sync.dma_start(out=agin_c[NBU_C * P:C_S, :], in_=zrow_sb[:])

            # gather windows: (row start, rows) into the direction's table
            WIN_V = [(w * VWROWS, VWROWS) for w in range(4)]
            WIN_C = [(w * CWROWS, CWROWS) for w in range(2)]

            # ---- initial embeddings hT = relu(W.T @ featT + b)
            def emit_init(featT_dram, fdim, n_s, w_sb, b_sb, hT):
                with tc.tile_pool(name="initp", bufs=2) as ipool:
                    c0 = 0
                    while c0 < n_s:
                        w = min(512, n_s - c0)
                        fch = ipool.tile([fdim, 512], F32, name="fch", tag="fch")
                        nc.sync.dma_start(out=fch[:, :w],
                                          in_=featT_dram[:, c0:c0 + w])
                        psi = ps_misc.tile([H, 512], F32, name="psi", tag="misc")
                        nc.tensor.matmul(out=psi[:, :w], lhsT=w_sb[:],
                                         rhs=fch[:, :w], start=True, stop=True)
                        nc.scalar.activation(out=hT[:, c0:c0 + w], in_=psi[:, :w],
                                             func=Act.Relu, bias=b_sb[:])
                        c0 += w

            emit_init(vfT, VF, V_S, wvar_sb, bvar_sb, hvT)
            emit_init(cfT, CF, C_S, wcon_sb, bcon_sb, hcT)

            qctr = [0]

            def emit_wb_dma(agin, rstage, b0, b1):
                nc.sync.dma_start(
                    out=agin[b0 * P:b1 * P, 0:H]
                    .rearrange("(b p) f -> p b f", p=P),
                    in_=rstage[:, b0 * H:b1 * H]
                    .rearrange("p (b f) -> p b f", f=H))

            def emit_ag(agin, tab):
                nc.gpsimd.collective_compute(
                    "AllGather", mybir.AluOpType.bypass,
                    replica_groups=[list(range(CORES))],
                    ins=[agin[:]], outs=[tab[:]])

            def emit_writeback(nblk, agin, tab, rstage):
                emit_wb_dma(agin, rstage, 0, nblk)
                emit_ag(agin, tab)

            def emit_shard_publish(hT, nblk, agin, tab):
                rstage = row_pool.tile([P, NBU_V * H], BF16, name="rstage",
                                       tag="rstage")
                for b in range(nblk):
                    psr = ps_misc.tile([P, H], BF16, name="psr", tag="misc")
                    nc.tensor.transpose(out=psr[:], in_=hT[:, b * P:(b + 1) * P],
                                        identity=ident_sb[:])
                    nc.vector.tensor_copy(out=rstage[:, b * H:(b + 1) * H],
                                          in_=psr[:])
                emit_writeback(nblk, agin, tab, rstage)

            emit_shard_publish(hvT, NBU_V, agin_v, tabs_v[0])

            # ---- one message-passing phase
            def emit_phase(tab_src, wins, idx_sb, pdst_sb, Tqw, d0a, swa,
                           qlist, hT, W_sb, b_sb, wb):
                nwin = Tqw.shape[1]
                grp_base = np.zeros((len(qlist), nwin), np.int64)
                grp_base.reshape(-1)[1:] = np.cumsum(Tqw.reshape(-1))[:-1]
                rstage = row_pool.tile([P, NBU_V * H], BF16, name="rstage",
                                       tag="rstage")
                qb = 0
                for qi, (chk, nb) in enumerate(qlist):
                    nmm = int(Tqw[qi].sum())
                    ps = ps_acc.tile([H, QW], F32, name="ps", tag="ps")
                    nc.tensor.matmul(out=ps[:, :nb * P], lhsT=zrow_sb[:, :H],
                                     rhs=szero_sb[:, :nb * P], start=True,
                                     stop=(nmm == 0))
                    mm = 0
                    for w in range(nwin):
                        Tg = int(Tqw[qi, w])
                        base = int(grp_base[qi, w])
                        r0, rws = wins[w]
                        done = 0
                        while done < Tg:
                            tc_ = min(TPC, Tg - done)
                            g = gpool.tile([P, TPC, H], BF16, name="g", tag="g")
                            _dma_gather_raw(
                                nc.gpsimd, g[:, :tc_, :],
                                tab_src[r0:r0 + rws, 0:H],
                                idx_sb[:, (base + done) * 8:
                                       (base + done + tc_) * 8],
                                num_idxs=tc_ * P, elem_size=H, elem_step=ROWB,
                                queue_num=qctr[0] % 4)
                            qctr[0] += 1
                            for t in range(tc_):
                                gt = base + done + t
                                d0_, sw_ = int(d0a[gt]), int(swa[gt])
                                s2 = s2_pool.tile([P, P], BF16, name="s2",
                                                  tag="s2")
                                nc.vector.tensor_tensor(
                                    out=s2[:, :sw_], in0=iota_sb[:, :sw_],
                                    in1=pdst_sb[:, gt:gt + 1]
                                    .to_broadcast([P, sw_]),
                                    op=AluOp.is_equal)
                                nc.tensor.matmul(
                                    out=ps[:, d0_:d0_ + sw_],
                                    lhsT=g[:, t, :], rhs=s2[:, :sw_],
                                    start=False, stop=(mm == nmm - 1))
                                mm += 1
                            done += tc_
                    # quad update: hT = relu(hT + W.T @ msgT + b)
                    cw = nb * P
                    msg = msg_pool.tile([H, QW], F32, name="msg", tag="msg")
                    nc.vector.tensor_copy(out=msg[:, :cw], in_=ps[:, :cw])
                    psu = ps_upd.tile([H, QW], F32, name="psu", tag="psu")
                    nc.tensor.matmul(out=psu[:, :cw], lhsT=W_sb[:],
                                     rhs=msg[:, :cw], start=True, stop=True)
                    tmp = msg_pool.tile([H, QW], F32, name="tmp", tag="tmp")
                    nc.vector.tensor_tensor(out=tmp[:, :cw], in0=psu[:, :cw],
                                            in1=hT[:, qb * P:qb * P + cw],
                                            op=AluOp.add)
                    nc.scalar.activation(out=hT[:, qb * P:qb * P + cw],
                                         in_=tmp[:, :cw],
                                         func=Act.Relu, bias=b_sb[:])
                    if wb is not None:
                        for j in range(nb):
                            b = qb + j
                            psr = ps_misc.tile([P, H], BF16, name="psr",
                                               tag="misc")
                            nc.tensor.transpose(
                                out=psr[:], in_=hT[:, b * P:(b + 1) * P],
                                identity=ident_sb[:])
                            nc.vector.tensor_copy(
                                out=rstage[:, b * H:(b + 1) * H], in_=psr[:])
                        emit_wb_dma(wb[0], rstage, qb, qb + nb)
                    qb += nb
                if wb is not None:
                    emit_ag(wb[0], wb[1])

            for r in range(ROUNDS):
                emit_phase(tabs_v[r], WIN_V, idx_v2c_sb, pdst_v2c_sb, Tqw_c,
                           d0_c, sw_c, QLIST_C, hcT, wv2c_sb, bv2c_sb,
                           (agin_c, tabs_c[r]))
                last = r == ROUNDS - 1
                emit_phase(tabs_c[r], WIN_C, idx_c2v_sb, pdst_c2v_sb, Tqw_v,
                           d0_v, sw_v, QLIST_V, hvT, wc2v_sb, bc2v_sb,
                           None if last else (agin_v, tabs_v[r + 1]))

            # ---- scores = h_var @ w_score + b_score (shard)
            c0 = 0
            while c0 < V_S:
                w = min(512, V_S - c0)
                pss = ps_misc.tile([1, 512], F32, name="pss", tag="misc")
                nc.tensor.matmul(out=pss[:, :w], lhsT=wsco_sb[:],
                                 rhs=hvT[:, c0:c0 + w], start=True, stop=True)
                sch = s2_pool.tile([1, 512], F32, name="sch", tag="sch")
                nc.vector.tensor_scalar(
                    out=sch[:, :w], in0=pss[:, :w],
                    scalar1=float(b_score_val), scalar2=None, op0=AluOp.add)
                nc.sync.dma_start(out=scores_out[None, c0:c0 + w],
                                  in_=sch[0:1, :w])
                c0 += 512

    nc.compile()
    return nc


_CACHE = {}


def kernel(**inputs):
    var_feat = np.asarray(inputs["var_feat"], np.float32)
    constr_feat = np.asarray(inputs["constr_feat"], np.float32)
    var_idx = np.asarray(inputs["var_idx"]).astype(np.int64)
    constr_idx = np.asarray(inputs["constr_idx"]).astype(np.int64)
    b_score_val = float(np.asarray(inputs["b_score"]).reshape(-1)[0])

    key = (var_idx.tobytes(), constr_idx.tobytes())
    if key in _CACHE:
        nc, idx_v, pdst_v, idx_c, pdst_c = _CACHE[key]
    else:
        # v2c: dst=constr, src=var (gathers V table)
        idx_v, pdst_v, Tqw_c, d0_c, sw_c = _prep_direction(
            constr_idx, var_idx, C_CORE, QLIST_C, _win_var, 4)
        # c2v: dst=var, src=constr (gathers C table)
        idx_c, pdst_c, Tqw_v, d0_v, sw_v = _prep_direction(
            var_idx, constr_idx, V_CORE, QLIST_V, _win_con, 2)
        nc = _build(Tqw_c, Tqw_v, d0_c, sw_c, d0_v, sw_v, b_score_val)
        _CACHE[key] = (nc, idx_v, pdst_v, idx_c, pdst_c)

    iota = np.broadcast_to(np.arange(P, dtype=np.float32),
                           (P, P)).astype(BF).copy()
    ident = np.eye(H, dtype=np.float32).astype(BF)

    vf_pad = np.zeros((CORES, V_S, VF), np.float32)
    vf_pad[:, :V_CORE] = var_feat.reshape(CORES, V_CORE, VF)
    cf_pad = np.zeros((CORES, C_S, CF), np.float32)
    cf_pad[:, :C_CORE] = constr_feat.reshape(CORES, C_CORE, CF)

    common = dict(
        wvar=np.ascontiguousarray(inputs["W_var"], dtype=np.float32),
        wcon=np.ascontiguousarray(inputs["W_con"], dtype=np.float32),
        wv2c=np.ascontiguousarray(inputs["W_v2c"], dtype=np.float32),
        wc2v=np.ascontiguousarray(inputs["W_c2v"], dtype=np.float32),
        wsco=np.ascontiguousarray(inputs["W_score"], dtype=np.float32).astype(BF),
        bvar=np.ascontiguousarray(inputs["b_var"], dtype=np.float32).reshape(H, 1),
        bcon=np.ascontiguousarray(inputs["b_con"], dtype=np.float32).reshape(H, 1),
        bv2c=np.ascontiguousarray(inputs["b_v2c"], dtype=np.float32).reshape(H, 1),
        bc2v=np.ascontiguousarray(inputs["b_c2v"], dtype=np.float32).reshape(H, 1),
        iota=iota, ident=ident,
    )
    in_maps = []
    for k in range(CORES):
        m = dict(common)
        m["vfT"] = np.ascontiguousarray(vf_pad[k].T)
        m["cfT"] = np.ascontiguousarray(cf_pad[k].T)
        m["idx_v2c"] = idx_v[k]
        m["pdst_v2c"] = pdst_v[k]
        m["idx_c2v"] = idx_c[k]
        m["pdst_c2v"] = pdst_c[k]
        in_maps.append(m)

    res = run_bass_kernel_spmd(nc, in_maps, list(range(CORES)))
    scores = np.concatenate([res.results[k]["scores"].reshape(-1)[:V_CORE]
                             for k in range(CORES)])
    return scores.astype(np.float32)


# revision 18
# speedup vs baseline: 1.0236x; 1.0115x over previous
"""Trainium2 Bass kernel: bipartite GNN message passing (BranchingGNN), 8-core SPMD.

Sharding: core k owns constraint rows [k*6250,(k+1)*6250) and variable rows
[k*12500,(k+1)*12500); each core processes all edges targeting its shard, so
messages need no cross-core reduction. Node tables live row-major in DRAM
(bf16 features in the first 128B of a 256B-strided row) and are re-broadcast
each phase by an AllGather of the updated shards.

Per phase (one message direction):
  - dsts grouped into quads (4 dst-blocks = 512 psum columns); edges sorted
    by (quad, src-window, dst) and cut into raw 128-edge tiles (up to 7 per
    gather call). No slot structure; per-core shortfalls are pad slots that
    re-fetch one of the call's own rows (spread, no hot line) with pdst=-1,
    so every descriptor is a real 128B row read (descriptor count ~= edges,
    vs ~1.3x with slot padding, and no hot dummy row serializing HBM).
  - dma_gather (custom emit: 128B rows at 256B stride) fetches source rows
    row-major [128 edges, 64] bf16.
  - per tile: DVE is_equal(iota, pdst) builds a narrow one-hot S
    [128, sw<=128] bf16 over the tile's dst span; one PE matmul (lhsT=g,
    rhs=S) accumulates into the quad's PSUM [64,512] msgT at column d0
    (the quad's psum is zero-initialized by one matmul against a zero S).
  - per quad: relu(h_prevT + W.T @ msgT + b) in transposed layout, PE
    transpose per block back to row-major into the writeback stage; after
    the last quad one DMA + AllGather republishes the updated shard.

The kernel is bound by SWDGE descriptor generation on the gpsimd queue
pairs (~9.6ns/idx ucode, 4 queue pairs in parallel); PE/DVE reduction and
the collectives fit underneath/between.
"""
import sys

sys.path.insert(0, "/opt/trn_rl_repo")

import numpy as np
import ml_dtypes

import concourse.bass as bass
import concourse.bacc as bacc
import concourse.mybir as mybir
import concourse.tile as tile
from concourse.bass_utils import run_bass_kernel_spmd

# ---- problem constants
V, C, E = 100000, 50000, 1250000
VF, CF, H = 32, 32, 64
ROUNDS = 3
CORES = 8
P = 128
QW = 512              # psum columns per quad (4 blocks)
TPC = 7               # tiles per gather call (SWDGE ring cap)
ROWB = 128            # table row width in bf16 elems (64 data + 64 pad = 256B)

V_CORE, C_CORE = 12500, 6250          # real nodes per core
V_S, C_S = 12672, 6400                # shard rows (99 / 50 blocks)
NBU_V, NBU_C = 98, 49                 # updated blocks (last block stays zero)
RV, RC = CORES * V_S, CORES * C_S     # 101376 / 51200 table rows
VWROWS, CWROWS = 2 * V_S, 4 * C_S     # 25344 / 25600 rows per window

BF16 = mybir.dt.bfloat16
F32 = mybir.dt.float32
I16 = mybir.dt.int16
BF = ml_dtypes.bfloat16

# quads: (chunk, nb). V-dst: chunk0 blocks 0..49, chunk1 50..97.
QLIST_V = [(0, 4)] * 12 + [(0, 2)] + [(1, 4)] * 12
# C-dst: chunk0 blocks 0..24, chunk1 25..48.
QLIST_C = [(0, 4)] * 6 + [(0, 1)] + [(1, 4)] * 6


def _win_var(src):
    """var id -> (window, window-local row): windows of 2 shards."""
    k = src // V_CORE
    l = src % V_CORE
    w = k // 2
    widx = (k % 2) * V_S + l
    return w, widx


def _win_con(src):
    """constr id -> (window, window-local row): windows of 4 shards."""
    k = src // C_CORE
    l = src % C_CORE
    w = k // 4
    widx = (k % 4) * C_S + l
    return w, widx


def _prep_direction(dst, src, n_dst_core, qlist, win_of, nwin):
    """Per-direction metadata. Edges sorted by (core, quad, window, dst);
    per (quad, window) group cut into raw 128-edge tiles; per-core pad
    slots re-fetch one of the call's own rows with pdst=-1.

    Returns (idx [CORES,128,T*8] i16, pdst_rel [CORES,128,T] bf16,
    Tqw [nq,nwin], d0 [T], sw [T])."""
    dst = np.asarray(dst, np.int64)
    src = np.asarray(src, np.int64)
    nq = len(qlist)
    qstart_blk = np.cumsum([0] + [nb for _, nb in qlist])
    blk2q = np.zeros(qstart_blk[-1], np.int64)
    for qi in range(nq):
        blk2q[qstart_blk[qi]:qstart_blk[qi + 1]] = qi

    core_of = dst // n_dst_core
    d_loc = dst % n_dst_core
    b_of = d_loc // P
    q_of = blk2q[b_of]
    dq = d_loc - qstart_blk[q_of] * P                # 0..nb*128-1
    w_of, widx = win_of(src)

    key = ((core_of * nq + q_of) * nwin + w_of) * QW + dq
    order = np.argsort(key, kind="stable")
    ks = key[order]
    widx_s = widx[order]
    dq_s = dq[order]

    gk = ks // QW                                    # (core, q, w) group id
    counts = np.bincount(gk, minlength=CORES * nq * nwin) \
        .reshape(CORES, nq, nwin)
    Tqw = -(-counts.max(0) // P)                     # [nq, nwin] max tiles

    grp_base = np.zeros((nq, nwin), np.int64)
    flat = Tqw.reshape(-1)
    grp_base.reshape(-1)[1:] = np.cumsum(flat)[:-1]
    Ttot = int(flat.sum())

    gcounts = np.bincount(gk, minlength=CORES * nq * nwin)
    gstart = np.zeros(gcounts.size + 1, np.int64)
    gstart[1:] = np.cumsum(gcounts)
    rank = np.arange(dst.size, dtype=np.int64) - gstart[gk]

    c_s = gk // (nq * nwin)
    qw = gk % (nq * nwin)
    epos = grp_base.reshape(-1)[qw] * P + rank

    idx16 = np.full((CORES, Ttot * P), -1, np.int16)
    idx16[c_s, epos] = widx_s.astype(np.int16)
    pdst = np.full((CORES, Ttot * P), -1.0, np.float32)
    pdst[c_s, epos] = dq_s.astype(np.float32)

    # fill pad slots: cycle each call's pads over its own valid idxs
    flatT = Tqw.reshape(-1)
    flat_base = grp_base.reshape(-1)
    for qwi in np.nonzero(flatT > 0)[0]:
        base, T = int(flat_base[qwi]), int(flatT[qwi])
        q_, w_ = qwi // nwin, qwi % nwin
        for j0 in range(0, T, TPC):
            clen = min(TPC, T - j0) * P
            cpos = (base + j0) * P
            for k in range(CORES):
                valid = min(max(int(counts[k, q_, w_]) - j0 * P, 0), clen)
                if valid == clen:
                    continue
                if valid > 0:
                    npad = clen - valid
                    reps = -(-npad // valid)
                    idx16[k, cpos + valid:cpos + clen] = \
                        np.tile(idx16[k, cpos:cpos + valid], reps)[:npad]
                else:
                    gbase = int(flat_base[qwi]) * P
                    gval = min(int(counts[k, q_, w_]), clen)
                    if gval == 0:
                        idx16[k, cpos:cpos + clen] = 0
                    else:
                        reps = -(-clen // gval)
                        idx16[k, cpos:cpos + clen] = \
                            np.tile(idx16[k, gbase:gbase + gval], reps)[:clen]

    # per-tile dst span [d0, d0+sw) in quad coordinates (shared across cores)
    pd3 = pdst.reshape(CORES, Ttot, P)
    valid = pd3 >= 0
    dmin = np.where(valid, pd3, 1e9).min(axis=(0, 2))
    dmax = np.where(valid, pd3, -1).max(axis=(0, 2))
    d0 = np.clip(dmin, 0, QW - 1).astype(np.int64)
    sw = np.maximum(dmax - d0 + 1, 1).astype(np.int64)
    assert sw.max() <= P, f"tile dst span {sw.max()} exceeds {P}"
    pdst_rel = pd3 - d0[None, :, None]
    pdst_rel[~valid] = -1
    pdst_rel = pdst_rel.transpose(0, 2, 1).astype(np.float32)  # [CORES,P,T]

    packed = np.zeros((CORES, P, Ttot * 8), np.int16)
    for k in range(CORES):
        a = idx16[k].reshape(-1, 16).T               # [16, Ttot*8]
        packed[k] = np.tile(a, (8, 1))
    return packed, pdst_rel.astype(BF), Tqw.astype(int), d0, sw


def _dma_gather_raw(gp, out_ap, in_ap, idxs_ap, num_idxs, elem_size, elem_step,
                    queue_num=0):
    """dma_gather (non-transpose, HBM source) allowing 128B rows at 256B stride."""
    from concourse import ap_utils
    gp._assert_queue_num(queue_num)
    assert idxs_ap.dtype == mybir.dt.int16
    assert in_ap.dtype == out_ap.dtype
    assert ap_utils.ap_is_contiguous(in_ap.ap[1:])
    assert ap_utils.ap_is_contiguous(out_ap.ap[1:])
    assert ap_utils.ap_is_contiguous(idxs_ap.ap[1:])
    assert in_ap.ap[-1][1] == out_ap.ap[-1][1] == elem_size
    assert out_ap.ap[0][1] * out_ap.ap[1][1] == num_idxs and num_idxs % 128 == 0
    assert in_ap.ap[0][0] == elem_step
    stride_bytes = elem_step * mybir.dt.size(in_ap.dtype)
    stride_bytes_256 = stride_bytes // 256
    assert stride_bytes_256 * 256 == stride_bytes and stride_bytes_256 < 256
    _in_ap = gp.lower_ap_dma(in_ap, for_custom_bir_dma=True)
    _idxs_ap = gp.lower_ap(idxs_ap)
    _out_ap = gp.lower_ap(out_ap)
    return gp.add_instruction(
        mybir.InstDMAGatherAnt(
            name=gp.bass.get_next_instruction_name(),
            ins=[*_in_ap, _idxs_ap, gp.lower_val_access(gp.to_reg(num_idxs))],
            outs=[_out_ap],
            transpose=False, num_idxs=num_idxs, elem_size=elem_size,
            stride_bytes_256=stride_bytes_256, gen_mode=0, single_packet=True,
            queue_num=queue_num, sbuf_tokens_per_rank=0,
            sbuf_free_dim_per_rank=0, sbuf_free_dim_pad_per_rank=0,
            sbuf_byte_offset=0))


def _build(Tqw_c, Tqw_v, d0_c, sw_c, d0_v, sw_v, b_score_val):
    """Build the shared SPMD program."""

    Tt_c, Tt_v = int(Tqw_c.sum()), int(Tqw_v.sum())

    nc = bacc.Bacc("TRN2", target_bir_lowering=False, num_devices=CORES,
                   num_swdge_queues=4)
    AluOp = mybir.AluOpType
    Act = mybir.ActivationFunctionType

    def ein(name, shape, dtype):
        return nc.dram_tensor(name, shape, dtype, kind="ExternalInput")

    vfT = ein("vfT", [VF, V_S], F32)
    cfT = ein("cfT", [CF, C_S], F32)
    wvar = ein("wvar", [VF, H], F32)
    wcon = ein("wcon", [CF, H], F32)
    wv2c = ein("wv2c", [H, H], F32)
    wc2v = ein("wc2v", [H, H], F32)
    wsco = ein("wsco", [H, 1], BF16)
    bvar = ein("bvar", [H, 1], F32)
    bcon = ein("bcon", [H, 1], F32)
    bv2c = ein("bv2c", [H, 1], F32)
    bc2v = ein("bc2v", [H, 1], F32)
    idx_v2c_d = ein("idx_v2c", [P, Tt_c * 8], I16)
    idx_c2v_d = ein("idx_c2v", [P, Tt_v * 8], I16)
    pdst_v2c_d = ein("pdst_v2c", [P, Tt_c], BF16)
    pdst_c2v_d = ein("pdst_c2v", [P, Tt_v], BF16)
    iota_d = ein("iota", [P, P], BF16)
    ident_d = ein("ident", [H, H], BF16)
    scores_out = nc.dram_tensor("scores", [V_S], F32, kind="ExternalOutput")

    with tile.TileContext(nc) as tc:
        with (
            tc.tile_pool(name="const", bufs=1) as cpool,
            tc.tile_pool(name="state", bufs=1) as spool,
            tc.tile_pool(name="dram", bufs=1, space="DRAM") as dpool,
            tc.tile_pool(name="gpool", bufs=24) as gpool,
            tc.tile_pool(name="s2p", bufs=6) as s2_pool,
            tc.tile_pool(name="msgp", bufs=2) as msg_pool,
            tc.tile_pool(name="rowp", bufs=2) as row_pool,
            tc.tile_pool(name="ps_acc", bufs=3, space="PSUM") as ps_acc,
            tc.tile_pool(name="ps_upd", bufs=2, space="PSUM") as ps_upd,
            tc.tile_pool(name="ps_misc", bufs=2, space="PSUM") as ps_misc,
        ):
            def load_const(name, dram, shape, dtype):
                t = cpool.tile(shape, dtype, name=name)
                nc.sync.dma_start(out=t[:], in_=dram[:])
                return t

            iota_sb = load_const("iota_sb", iota_d, [P, P], BF16)
            ident_sb = load_const("ident_sb", ident_d, [H, H], BF16)
            wvar_sb = load_const("wvar_sb", wvar, [VF, H], F32)
            wcon_sb = load_const("wcon_sb", wcon, [CF, H], F32)
            wv2c_sb = load_const("wv2c_sb", wv2c, [H, H], F32)
            wc2v_sb = load_const("wc2v_sb", wc2v, [H, H], F32)
            wsco_sb = load_const("wsco_sb", wsco, [H, 1], BF16)
            bvar_sb = load_const("bvar_sb", bvar, [H, 1], F32)
            bcon_sb = load_const("bcon_sb", bcon, [H, 1], F32)
            bv2c_sb = load_const("bv2c_sb", bv2c, [H, 1], F32)
            bc2v_sb = load_const("bc2v_sb", bc2v, [H, 1], F32)
            idx_v2c_sb = load_const("idx_v2c_sb", idx_v2c_d, [P, Tt_c * 8], I16)
            idx_c2v_sb = load_const("idx_c2v_sb", idx_c2v_d, [P, Tt_v * 8], I16)
            pdst_v2c_sb = load_const("pdst_v2c_sb", pdst_v2c_d, [P, Tt_c], BF16)
            pdst_c2v_sb = load_const("pdst_c2v_sb", pdst_c2v_d, [P, Tt_v], BF16)

            zrow_sb = cpool.tile([P, ROWB], BF16, name="zrow_sb")
            nc.vector.memset(zrow_sb[:], 0.0)
            szero_sb = cpool.tile([P, QW], BF16, name="szero_sb")
            nc.vector.memset(szero_sb[:], 0.0)

            hvT = spool.tile([H, V_S], BF16, name="hvT")
            hcT = spool.tile([H, C_S], BF16, name="hcT")
            tabs_v = [dpool.tile([RV, ROWB], BF16, name=f"tab_v{i}",
                                 addr_space="Shared", tag=f"tab_v{i}")
                      for i in range(ROUNDS)]
            tabs_c = [dpool.tile([RC, ROWB], BF16, name=f"tab_c{i}",
                                 addr_space="Shared", tag=f"tab_c{i}")
                      for i in range(ROUNDS)]
            agin_v = dpool.tile([V_S, ROWB], BF16, name="agin_v")
            agin_c = dpool.tile([C_S, ROWB], BF16, name="agin_c")

            # zero the shard tail (pad rows shipped by every AllGather)
            nc.sync.dma_start(out=agin_v[NBU_V * P:V_S, :], in_=zrow_sb[:])
            nc.sync.dma_start(out=agin_c[NBU_C * P:C_S, :], in_=zrow_sb[:])

            # gather windows: (row start, rows) into the direction's table
            WIN_V = [(w * VWROWS, VWROWS) for w in range(4)]
            WIN_C = [(w * CWROWS, CWROWS) for w in range(2)]

            # ---- initial embeddings hT = relu(W.T @ featT + b)
            def emit_init(featT_dram, fdim, n_s, w_sb, b_sb, hT):
                with tc.tile_pool(name="initp", bufs=2) as ipool:
                    c0 = 0
                    while c0 < n_s:
                        w = min(512, n_s - c0)
                        fch = ipool.tile([fdim, 512], F32, name="fch", tag="fch")
                        nc.sync.dma_start(out=fch[:, :w],
                                          in_=featT_dram[:, c0:c0 + w])
                        psi = ps_misc.tile([H, 512], F32, name="psi", tag="misc")
                        nc.tensor.matmul(out=psi[:, :w], lhsT=w_sb[:],
                                         rhs=fch[:, :w], start=True, stop=True)
                        nc.scalar.activation(out=hT[:, c0:c0 + w], in_=psi[:, :w],
                                             func=Act.Relu, bias=b_sb[:])
                        c0 += w

            emit_init(vfT, VF, V_S, wvar_sb, bvar_sb, hvT)

            qctr = [0]

            def emit_wb_dma(agin, rstage, b0, b1):
                nc.sync.dma_start(
                    out=agin[b0 * P:b1 * P, 0:H]
                    .rearrange("(b p) f -> p b f", p=P),
                    in_=rstage[:, b0 * H:b1 * H]
                    .rearrange("p (b f) -> p b f", f=H))

            def emit_ag(agin, tab):
                nc.gpsimd.collective_compute(
                    "AllGather", mybir.AluOpType.bypass,
                    replica_groups=[list(range(CORES))],
                    ins=[agin[:]], outs=[tab[:]])

            def emit_writeback(nblk, agin, tab, rstage):
                emit_wb_dma(agin, rstage, 0, nblk)
                emit_ag(agin, tab)

            def emit_shard_publish(hT, nblk, agin, tab):
                rstage = row_pool.tile([P, NBU_V * H], BF16, name="rstage",
                                       tag="rstage")
                for b in range(nblk):
                    psr = ps_misc.tile([P, H], BF16, name="psr", tag="misc")
                    nc.tensor.transpose(out=psr[:], in_=hT[:, b * P:(b + 1) * P],
                                        identity=ident_sb[:])
                    nc.vector.tensor_copy(out=rstage[:, b * H:(b + 1) * H],
                                          in_=psr[:])
                emit_writeback(nblk, agin, tab, rstage)

            emit_shard_publish(hvT, NBU_V, agin_v, tabs_v[0])
            emit_init(cfT, CF, C_S, wcon_sb, bcon_sb, hcT)

            # ---- one message-passing phase
            def emit_phase(tab_src, wins, idx_sb, pdst_sb, Tqw, d0a, swa,
                           qlist, hT, W_sb, b_sb, wb):
                nwin = Tqw.shape[1]
                grp_base = np.zeros((len(qlist), nwin), np.int64)
                grp_base.reshape(-1)[1:] = np.cumsum(Tqw.reshape(-1))[:-1]
                rstage = row_pool.tile([P, NBU_V * H], BF16, name="rstage",
                                       tag="rstage")
                qb = 0
                for qi, (chk, nb) in enumerate(qlist):
                    nmm = int(Tqw[qi].sum())
                    ps = ps_acc.tile([H, QW], F32, name="ps", tag="ps")
                    nc.tensor.matmul(out=ps[:, :nb * P], lhsT=zrow_sb[:, :H],
                                     rhs=szero_sb[:, :nb * P], start=True,
                                     stop=(nmm == 0))
                    mm = 0
                    for w in range(nwin):
                        Tg = int(Tqw[qi, w])
                        base = int(grp_base[qi, w])
                        r0, rws = wins[w]
                        done = 0
                        while done < Tg:
                            tc_ = min(TPC, Tg - done)
                            g = gpool.tile([P, TPC, H], BF16, name="g", tag="g")
                            _dma_gather_raw(
                                nc.gpsimd, g[:, :tc_, :],
                                tab_src[r0:r0 + rws, 0:H],
                                idx_sb[:, (base + done) * 8:
                                       (base + done + tc_) * 8],
                                num_idxs=tc_ * P, elem_size=H, elem_step=ROWB,
                                queue_num=qctr[0] % 4)
                            qctr[0] += 1
                            for t in range(tc_):
                                gt = base + done + t
                                d0_, sw_ = int(d0a[gt]), int(swa[gt])
                                s2 = s2_pool.tile([P, P], BF16, name="s2",
                                                  tag="s2")
                                nc.vector.tensor_tensor(
                                    out=s2[:, :sw_], in0=iota_sb[:, :sw_],
                                    in1=pdst_sb[:, gt:gt + 1]
                                    .to_broadcast([P, sw_]),
                                    op=AluOp.is_equal)
                                nc.tensor.matmul(
                                    out=ps[:, d0_:d0_ + sw_],
                                    lhsT=g[:, t, :], rhs=s2[:, :sw_],
                                    start=False, stop=(mm == nmm - 1))
                                mm += 1
                            done += tc_
                    # quad update: hT = relu(hT + W.T @ msgT + b)
                    cw = nb * P
                    msg = msg_pool.tile([H, QW], F32, name="msg", tag="msg")
                    nc.vector.tensor_copy(out=msg[:, :cw], in_=ps[:, :cw])
                    psu = ps_upd.tile([H, QW], F32, name="psu", tag="psu")
                    nc.tensor.matmul(out=psu[:, :cw], lhsT=W_sb[:],
                                     rhs=msg[:, :cw], start=True, stop=True)
                    tmp = msg_pool.tile([H, QW], F32, name="tmp", tag="tmp")
                    nc.vector.tensor_tensor(out=tmp[:, :cw], in0=psu[:, :cw],
                                            in1=hT[:, qb * P:qb * P + cw],
                                            op=AluOp.add)
                    nc.scalar.activation(out=hT[:, qb * P:qb * P + cw],
                                         in_=tmp[:, :cw],
                                         func=Act.Relu, bias=b_sb[:])
                    if wb is not None:
                        for j in range(nb):
                            b = qb + j
                            psr = ps_misc.tile([P, H], BF16, name="psr",
                                               tag="misc")
                            nc.tensor.transpose(
                                out=psr[:], in_=hT[:, b * P:(b + 1) * P],
                                identity=ident_sb[:])
                            nc.vector.tensor_copy(
                                out=rstage[:, b * H:(b + 1) * H], in_=psr[:])
                        emit_wb_dma(wb[0], rstage, qb, qb + nb)
                    qb += nb
                if wb is not None:
                    emit_ag(wb[0], wb[1])

            for r in range(ROUNDS):
                emit_phase(tabs_v[r], WIN_V, idx_v2c_sb, pdst_v2c_sb, Tqw_c,
                           d0_c, sw_c, QLIST_C, hcT, wv2c_sb, bv2c_sb,
                           (agin_c, tabs_c[r]))
                last = r == ROUNDS - 1
                emit_phase(tabs_c[r], WIN_C, idx_c2v_sb, pdst_c2v_sb, Tqw_v,
                           d0_v, sw_v, QLIST_V, hvT, wc2v_sb, bc2v_sb,
                           None if last else (agin_v, tabs_v[r + 1]))

            # ---- scores = h_var @ w_score + b_score (shard)
            c0 = 0
            while c0 < V_S:
                w = min(512, V_S - c0)
                pss = ps_misc.tile([1, 512], F32, name="pss", tag="misc")
                nc.tensor.matmul(out=pss[:, :w], lhsT=wsco_sb[:],
                                 rhs=hvT[:, c0:c0 + w], start=True, stop=True)
                sch = s2_pool.tile([1, 512], F32, name="sch", tag="sch")
                nc.vector.tensor_scalar(
                    out=sch[:, :w], in0=pss[:, :w],
                    scalar1=float(b_score_val), scalar2=None, op0=AluOp.add)
                nc.sync.dma_start(out=scores_out[None, c0:c0 + w],
                                  in_=sch[0:1, :w])
                c0 += 512

    nc.compile()
    return nc


_CACHE = {}


def kernel(**inputs):
    var_feat = np.asarray(inputs["var_feat"], np.float32)
    constr_feat = np.asarray(inputs["constr_feat"], np.float32)
    var_idx = np.asarray(inputs["var_idx"]).astype(np.int64)
    constr_idx = np.asarray(inputs["constr_idx"]).astype(np.int64)
    b_score_val = float(np.asarray(inputs["b_score"]).reshape(-1)[0])

    key = (var_idx.tobytes(), constr_idx.tobytes())
    if key in _CACHE:
        nc, idx_v, pdst_v, idx_c, pdst_c = _CACHE[key]
    else:
        # v2c: dst=constr, src=var (gathers V table)
        idx_v, pdst_v, Tqw_c, d0_c, sw_c = _prep_direction(
            constr_idx, var_idx, C_CORE, QLIST_C, _win_var, 4)
        # c2v: dst=var, src=constr (gathers C table)
        idx_c, pdst_c, Tqw_v, d0_v, sw_v = _prep_direction(
            var_idx, constr_idx, V_CORE, QLIST_V, _win_con, 2)
        nc = _build(Tqw_c, Tqw_v, d0_c, sw_c, d0_v, sw_v, b_score_val)
        _CACHE[key] = (nc, idx_v, pdst_v, idx_c, pdst_c)

    iota = np.broadcast_to(np.arange(P, dtype=np.float32),
                           (P, P)).astype(BF).copy()
    ident = np.eye(H, dtype=np.float32).astype(BF)

    vf_pad = np.zeros((CORES, V_S, VF), np.float32)
    vf_pad[:, :V_CORE] = var_feat.reshape(CORES, V_CORE, VF)
    cf_pad = np.zeros((CORES, C_S, CF), np.float32)
    cf_pad[:, :C_CORE] = constr_feat.reshape(CORES, C_CORE, CF)

    common = dict(
        wvar=np.ascontiguousarray(inputs["W_var"], dtype=np.float32),
        wcon=np.ascontiguousarray(inputs["W_con"], dtype=np.float32),
        wv2c=np.ascontiguousarray(inputs["W_v2c"], dtype=np.float32),
        wc2v=np.ascontiguousarray(inputs["W_c2v"], dtype=np.float32),
        wsco=np.ascontiguousarray(inputs["W_score"], dtype=np.float32).astype(BF),
        bvar=np.ascontiguousarray(inputs["b_var"], dtype=np.float32).reshape(H, 1),
        bcon=np.ascontiguousarray(inputs["b_con"], dtype=np.float32).reshape(H, 1),
        bv2c=np.ascontiguousarray(inputs["b_v2c"], dtype=np.float32).reshape(H, 1),
        bc2v=np.ascontiguousarray(inputs["b_c2v"], dtype=np.float32).reshape(H, 1),
        iota=iota, ident=ident,
    )
    in_maps = []
    for k in range(CORES):
        m = dict(common)
        m["vfT"] = np.ascontiguousarray(vf_pad[k].T)
        m["cfT"] = np.ascontiguousarray(cf_pad[k].T)
        m["idx_v2c"] = idx_v[k]
        m["pdst_v2c"] = pdst_v[k]
        m["idx_c2v"] = idx_c[k]
        m["pdst_c2v"] = pdst_c[k]
        in_maps.append(m)

    res = run_bass_kernel_spmd(nc, in_maps, list(range(CORES)))
    scores = np.concatenate([res.results[k]["scores"].reshape(-1)[:V_CORE]
                             for k in range(CORES)])
    return scores.astype(np.float32)
